# revision 19
# baseline (speedup 1.0000x reference)
"""GaborNet Trainium2 kernel.

Math: per pixel p=(x1,x2) (the 2 input channels), per layer l, channel c:
  exp-arg  q_lc(p) = -0.5*||diag(gamma) R (p-mu)||^2   (quadratic in x1,x2)
  sin-arg  s_lc(p) = filt_w . p + filt_b               (linear)
  g_l = exp(q) * sin(s);  out_0 = g_0
  out_l = g_l * (W_{l-1} @ out_{l-1} + b_{l-1});  final = out_w @ out_4 + out_b

The quadratic form is expanded into 5 shared per-pixel features
[x1, x2, x1^2, x2^2, x1*x2] so that all per-channel math becomes matmuls
(PE), exp/sin run on the scalar engine (ACT), and only cheap elementwise
multiplies remain on the vector engine (DVE).

Layout: channels on partitions, pixels on the free dim. Tiles of T=2048
pixels are split into two groups of C=1024 stacked on partitions
(64ch x 2 groups = 128 partitions) via block-diagonal lhsT packing.

Exp and Sin live in different ACT table sets (~2.7us per switch), so
tiles are processed in super-tiles of 4: all Exp activations first, then
all Sin (+ MLP) work.

Sharding: 8 cores x 65536 consecutive pixels (batch-major, then rows).
"""

import numpy as np

B, DIM, H, W = 2, 2, 512, 512
HID, OUT, NL = 64, 3, 4
NCORES = 8
NPIX = B * H * W // NCORES  # 65536 pixels per core
T = 2048                    # pixels per tile
C = T // 2                  # packed columns (2 pixel-groups on partitions)
NT = NPIX // T              # 32 tiles
ST = 4                      # tiles per super-tile (ACT table-switch batching)
MMCHUNK = 512               # fp32 moving-operand limit per matmul

_CACHE = {}


def _gabor_coeffs(filt_w, filt_b, mu, gamma, theta):
    """Host-side: per layer, coefficients of the exp-arg quadratic and the
    sin-arg linear on features [x1, x2, x1^2, x2^2, x1*x2], plus biases."""
    NL1 = theta.shape[0]
    Ge = np.zeros((NL1, 5, HID), np.float64)
    Gs = np.zeros((NL1, 5, HID), np.float64)
    be = np.zeros((NL1, HID), np.float64)
    bs = np.zeros((NL1, HID), np.float64)
    for l in range(NL1):
        ang = 2.0 * np.pi * theta[l].astype(np.float64)
        c, s = np.cos(ang), np.sin(ang)
        R = np.stack([np.stack([c, s], -1), np.stack([-s, c], -1)], -2)  # [64,2,2]
        A = gamma[l].astype(np.float64)[:, :, None] * R
        Q = np.einsum('coi,coj->cij', A, A)
        Qmu = np.einsum('cij,cj->ci', Q, mu[l].astype(np.float64))
        Ge[l, 0] = Qmu[:, 0]
        Ge[l, 1] = Qmu[:, 1]
        Ge[l, 2] = -0.5 * Q[:, 0, 0]
        Ge[l, 3] = -0.5 * Q[:, 1, 1]
        Ge[l, 4] = -Q[:, 0, 1]
        be[l] = -0.5 * np.einsum('ci,ci->c', mu[l].astype(np.float64), Qmu)
        Gs[l, 0] = filt_w[l, :, 0]
        Gs[l, 1] = filt_w[l, :, 1]
        bs[l] = filt_b[l]
    return Ge, Gs, be, bs


def _build_consts(filt_w, filt_b, mu, gamma, theta, lin_w, lin_b, out_w, out_b):
    Ge, Gs, be, bs = _gabor_coeffs(filt_w, filt_b, mu, gamma, theta)
    NL1 = NL + 1
    # gabor lhsT blocks: blocks 0..4 = exp layer l, 5..9 = sin layer l.
    # K rows 0-4: group A feats [x1, x2, x1^2, x2^2, x1x2]; rows 5-9: group B.
    gab = np.zeros((10, 10 * 128), np.float32)
    for l in range(NL1):
        for blk, G in ((l, Ge[l]), (5 + l, Gs[l])):
            gab[0:5, blk * 128:blk * 128 + 64] = G
            gab[5:10, blk * 128 + 64:blk * 128 + 128] = G
    # mlp lhsT blocks: diag(W^T, W^T)
    mlp = np.zeros((128, 4 * 128), np.float32)
    for l in range(NL):
        wT = lin_w[l].T.astype(np.float32)  # [in k, out m]
        mlp[0:64, l * 128:l * 128 + 64] = wT
        mlp[64:128, l * 128 + 64:l * 128 + 128] = wT
    # final lhsT: diag(out_w^T, out_w^T) -> [128, 6]
    fin = np.zeros((128, 6), np.float32)
    fin[0:64, 0:3] = out_w.T
    fin[64:128, 3:6] = out_w.T
    eb = np.concatenate([be, be], axis=1).T.astype(np.float32)    # [128, 5]
    sb = np.concatenate([bs, bs], axis=1).T.astype(np.float32)    # [128, 5]
    mb = np.concatenate([lin_b, lin_b], axis=1).T.astype(np.float32)  # [128, 4]
    ob = np.concatenate([out_b, out_b]).reshape(1, 6).astype(np.float32)
    ones = np.ones((1, C), np.float32)
    return dict(gab=gab, mlp=mlp, fin=fin, eb=eb, sb=sb, mb=mb, ob=ob, ones=ones)


def _build_nc():
    import concourse.mybir as mybir
    import concourse.tile as tile
    from concourse import bacc

    f32 = mybir.dt.float32
    f32r = mybir.dt.float32r
    AF = mybir.ActivationFunctionType
    ALU = mybir.AluOpType

    nc = bacc.Bacc("TRN2", target_bir_lowering=False, debug=False,
                   enable_asserts=False, num_devices=NCORES)

    xs = nc.dram_tensor("xs", [2, NPIX], f32r, kind="ExternalInput").ap()
    gab_d = nc.dram_tensor("gab", [10, 10 * 128], f32r, kind="ExternalInput").ap()
    mlp_d = nc.dram_tensor("mlp", [128, 4 * 128], f32r, kind="ExternalInput").ap()
    fin_d = nc.dram_tensor("fin", [128, 6], f32r, kind="ExternalInput").ap()
    eb_d = nc.dram_tensor("eb", [128, 5], f32, kind="ExternalInput").ap()
    sb_d = nc.dram_tensor("sb", [128, 5], f32, kind="ExternalInput").ap()
    mb_d = nc.dram_tensor("mb", [128, 4], f32, kind="ExternalInput").ap()
    ob_d = nc.dram_tensor("ob", [1, 6], f32r, kind="ExternalInput").ap()
    ones_d = nc.dram_tensor("ones", [1, C], f32r, kind="ExternalInput").ap()
    out_d = nc.dram_tensor("out", [3, NPIX], f32, kind="ExternalOutput").ap()

    def mm_pair(psum_ap, lhsT_ap, rhs_ap):
        for h in range(C // MMCHUNK):
            sl = slice(h * MMCHUNK, (h + 1) * MMCHUNK)
            nc.tensor.matmul(out=psum_ap[:, sl],
                             lhsT=lhsT_ap, rhs=rhs_ap[:, sl],
                             start=True, stop=True)

    with tile.TileContext(nc) as tc:
        with (
            tc.tile_pool(name="consts", bufs=1) as cpool,
            tc.tile_pool(name="feat", bufs=2) as fpool,
            tc.tile_pool(name="scrp", bufs=1) as scrpool,
            tc.tile_pool(name="pg", bufs=2, space="PSUM") as pg,
            tc.tile_pool(name="pm", bufs=2, space="PSUM") as pm,
            tc.tile_pool(name="epool", bufs=5 * ST) as epool,
            tc.tile_pool(name="spool", bufs=2) as spool,
            tc.tile_pool(name="gpool", bufs=5) as gpool,
            tc.tile_pool(name="opool", bufs=4) as opool,
            tc.tile_pool(name="obuf", bufs=3) as obpool,
        ):
            gab = cpool.tile([10, 10 * 128], f32r)
            nc.sync.dma_start(out=gab, in_=gab_d)
            mlp = cpool.tile([128, 4 * 128], f32r)
            nc.sync.dma_start(out=mlp, in_=mlp_d)
            fin = cpool.tile([128, 6], f32r)
            nc.sync.dma_start(out=fin, in_=fin_d)
            eb = cpool.tile([128, 5], f32)
            nc.sync.dma_start(out=eb, in_=eb_d)
            sb = cpool.tile([128, 5], f32)
            nc.sync.dma_start(out=sb, in_=sb_d)
            mb = cpool.tile([128, 4], f32)
            nc.sync.dma_start(out=mb, in_=mb_d)
            ob = cpool.tile([1, 6], f32r)
            nc.sync.dma_start(out=ob, in_=ob_d)
            ones = cpool.tile([1, C], f32r)
            nc.sync.dma_start(out=ones, in_=ones_d)

            pending = []

            def flush_finals():
                for t, cur in pending:
                    pf = pm.tile([128, C], f32, tag="lin")
                    for h in range(C // MMCHUNK):
                        sl = slice(h * MMCHUNK, (h + 1) * MMCHUNK)
                        nc.tensor.matmul(out=pf[0:6, sl], lhsT=ob,
                                         rhs=ones[:, sl], start=True,
                                         stop=False)
                        nc.tensor.matmul(out=pf[0:6, sl], lhsT=fin,
                                         rhs=cur[:, sl], start=False,
                                         stop=True)
                    osb = obpool.tile([6, C], f32, tag="osb")
                    # Identity lives in the exp table set; flushed during the
                    # Exp phase so no extra ACT table switches occur.
                    nc.scalar.activation(out=osb, in_=pf[0:6],
                                         func=AF.Identity, bias=0.0)
                    nc.sync.dma_start(out=out_d[:, t * T:t * T + C],
                                      in_=osb[0:3])
                    nc.sync.dma_start(out=out_d[:, t * T + C:(t + 1) * T],
                                      in_=osb[3:6])
                pending.clear()

            for st in range(NT // ST):
                # --- features for the whole super-tile ---------------------
                # Compute ops require all operands to start on the same
                # partition. Scratch tile: partitions = pixel group (A,B),
                # free dims = (feature j, col): j 0:x1 1:x2 2:x1^2 3:x2^2
                # 4:x1x2. Products are computed in-tile at partition 0, then
                # two contiguous SBUF->SBUF DMAs build the [10, C] K-block
                # per tile (rows 0-4 = A feats, 5-9 = B feats).
                feat = fpool.tile([10, ST, C], f32r)
                scr = []
                for tt in range(ST):
                    t = st * ST + tt
                    s2 = scrpool.tile([2, 5, C], f32r, tag="scr")
                    xtv = xs[:, t * T:(t + 1) * T]
                    xtv = xtv.rearrange("c (g p) -> g c p", p=C)
                    nc.sync.dma_start(out=s2[:, 0:2], in_=xtv)
                    nc.vector.tensor_mul(out=s2[:, 2:4], in0=s2[:, 0:2],
                                         in1=s2[:, 0:2])
                    nc.vector.tensor_mul(out=s2[:, 4], in0=s2[:, 0],
                                         in1=s2[:, 1])
                    nc.sync.dma_start(out=feat[0:5, tt],
                                      in_=s2[0:1].rearrange("p f c -> p (f c)"))
                    nc.sync.dma_start(out=feat[5:10, tt],
                                      in_=s2[1:2].rearrange("p f c -> p (f c)"))
                    scr.append(s2)

                # --- all Exp activations (one ACT table set) ---------------
                es = {}
                for tt in range(ST):
                    rhs = feat[0:10, tt]
                    for l in range(5):
                        ps = pg.tile([128, C], f32, tag="parg")
                        mm_pair(ps, gab[:, l * 128:(l + 1) * 128], rhs)
                        e = epool.tile([128, C], f32, tag="e")
                        nc.scalar.activation(out=e, in_=ps, func=AF.Exp,
                                             bias=eb[:, l:l + 1])
                        es[(tt, l)] = e

                flush_finals()

                # --- Sin + gabor product + MLP chain per tile --------------
                for tt in range(ST):
                    t = st * ST + tt
                    rhs = feat[0:10, tt]
                    g_tiles = []
                    for l in range(5):
                        ps = pg.tile([128, C], f32, tag="parg")
                        mm_pair(ps, gab[:, (5 + l) * 128:(6 + l) * 128], rhs)
                        s = spool.tile([128, C], f32, tag="s")
                        nc.scalar.activation(out=s, in_=ps, func=AF.Sin,
                                             bias=sb[:, l:l + 1])
                        g = gpool.tile([128, C], f32r, tag="g")
                        nc.vector.tensor_mul(out=g, in0=es[(tt, l)], in1=s)
                        g_tiles.append(g)

                    cur = g_tiles[0]
                    for l in range(1, 5):
                        pl = pm.tile([128, C], f32, tag="lin")
                        mm_pair(pl, mlp[:, (l - 1) * 128:l * 128], cur)
                        nxt = opool.tile([128, C], f32r, tag="o")
                        nc.vector.scalar_tensor_tensor(
                            out=nxt, in0=pl, scalar=mb[:, l - 1:l],
                            in1=g_tiles[l], op0=ALU.add, op1=ALU.mult)
                        cur = nxt

                    pending.append((t, cur))

            flush_finals()
    nc.compile()
    return nc


def _get_nc():
    if "nc" not in _CACHE:
        _CACHE["nc"] = _build_nc()
    return _CACHE["nc"]


def _in_maps(x, consts):
    maps = []
    rows = H // (NCORES // B)  # 128 rows per core
    for k in range(NCORES):
        b, r = k // (NCORES // B), (k % (NCORES // B)) * rows
        m = {"xs": np.ascontiguousarray(
            x[b, :, r:r + rows, :].reshape(2, NPIX), np.float32)}
        m.update(consts)
        maps.append(m)
    return maps


def _assemble(results):
    rows = H // (NCORES // B)
    out = np.empty((B, OUT, H, W), np.float32)
    for k in range(NCORES):
        b, r = k // (NCORES // B), (k % (NCORES // B)) * rows
        out[b, :, r:r + rows, :] = results[k]["out"].reshape(OUT, rows, W)
    return out


def run(x, filt_w, filt_b, mu, gamma, theta, lin_w, lin_b, out_w, out_b,
        trace=False):
    from concourse.bass_utils import run_bass_kernel_spmd
    nc = _get_nc()
    consts = _build_consts(np.asarray(filt_w), np.asarray(filt_b),
                           np.asarray(mu), np.asarray(gamma),
                           np.asarray(theta), np.asarray(lin_w),
                           np.asarray(lin_b), np.asarray(out_w),
                           np.asarray(out_b))
    maps = _in_maps(np.asarray(x), consts)
    res = run_bass_kernel_spmd(nc, maps, core_ids=list(range(NCORES)),
                               trace=trace)
    return _assemble(res.results), res


def kernel(**inputs):
    out, _ = run(**inputs)
    return out


# revision 20
# speedup vs baseline: 1.2276x; 1.2276x over previous
"""GaborNet Trainium2 kernel.

Math: per pixel p=(x1,x2) (the 2 input channels), per layer l, channel c:
  exp-arg  q_lc(p) = -0.5*||diag(gamma) R (p-mu)||^2   (quadratic in x1,x2)
  sin-arg  s_lc(p) = filt_w . p + filt_b               (linear)
  g_l = exp(q) * sin(s);  out_0 = g_0
  out_l = g_l * (W_{l-1} @ out_{l-1} + b_{l-1});  final = out_w @ out_4 + out_b

The quadratic form is expanded into 5 shared per-pixel features
[x1, x2, x1^2, x2^2, x1*x2] so that all per-channel math becomes matmuls
(PE), exp/sin run on the scalar engine (ACT), and only cheap elementwise
multiplies remain on the vector engine (DVE).

Layout: channels on partitions, pixels on the free dim. Tiles of T=2048
pixels are split into two groups of C=1024 stacked on partitions
(64ch x 2 groups = 128 partitions) via block-diagonal lhsT packing.

Exp and Sin live in different ACT table sets (~2.7us per switch), so
tiles are processed in super-tiles of 4: all Exp activations first, then
all Sin (+ MLP) work.

Sharding: 8 cores x 65536 consecutive pixels (batch-major, then rows).
"""

import numpy as np

B, DIM, H, W = 2, 2, 512, 512
HID, OUT, NL = 64, 3, 4
NCORES = 8
NPIX = B * H * W // NCORES  # 65536 pixels per core
T = 2048                    # pixels per tile
C = T // 2                  # packed columns (2 pixel-groups on partitions)
NT = NPIX // T              # 32 tiles
ST = 2                      # tiles per super-tile (ACT table-switch batching)
MMCHUNK = 512               # fp32 moving-operand limit per matmul

_CACHE = {}


def _gabor_coeffs(filt_w, filt_b, mu, gamma, theta):
    """Host-side: per layer, coefficients of the exp-arg quadratic and the
    sin-arg linear on features [x1, x2, x1^2, x2^2, x1*x2], plus biases."""
    NL1 = theta.shape[0]
    Ge = np.zeros((NL1, 5, HID), np.float64)
    Gs = np.zeros((NL1, 5, HID), np.float64)
    be = np.zeros((NL1, HID), np.float64)
    bs = np.zeros((NL1, HID), np.float64)
    for l in range(NL1):
        ang = 2.0 * np.pi * theta[l].astype(np.float64)
        c, s = np.cos(ang), np.sin(ang)
        R = np.stack([np.stack([c, s], -1), np.stack([-s, c], -1)], -2)  # [64,2,2]
        A = gamma[l].astype(np.float64)[:, :, None] * R
        Q = np.einsum('coi,coj->cij', A, A)
        Qmu = np.einsum('cij,cj->ci', Q, mu[l].astype(np.float64))
        Ge[l, 0] = Qmu[:, 0]
        Ge[l, 1] = Qmu[:, 1]
        Ge[l, 2] = -0.5 * Q[:, 0, 0]
        Ge[l, 3] = -0.5 * Q[:, 1, 1]
        Ge[l, 4] = -Q[:, 0, 1]
        be[l] = -0.5 * np.einsum('ci,ci->c', mu[l].astype(np.float64), Qmu)
        Gs[l, 0] = filt_w[l, :, 0]
        Gs[l, 1] = filt_w[l, :, 1]
        bs[l] = filt_b[l]
    return Ge, Gs, be, bs


def _build_consts(filt_w, filt_b, mu, gamma, theta, lin_w, lin_b, out_w, out_b):
    Ge, Gs, be, bs = _gabor_coeffs(filt_w, filt_b, mu, gamma, theta)
    NL1 = NL + 1
    # gabor lhsT blocks: blocks 0..4 = exp layer l, 5..9 = sin layer l.
    # K rows 0-4: group A feats [x1, x2, x1^2, x2^2, x1x2]; rows 5-9: group B.
    gab = np.zeros((10, 10 * 128), np.float32)
    for l in range(NL1):
        for blk, G in ((l, Ge[l]), (5 + l, Gs[l])):
            gab[0:5, blk * 128:blk * 128 + 64] = G
            gab[5:10, blk * 128 + 64:blk * 128 + 128] = G
    # mlp lhsT blocks: diag(W^T, W^T)
    mlp = np.zeros((128, 4 * 128), np.float32)
    for l in range(NL):
        wT = lin_w[l].T.astype(np.float32)  # [in k, out m]
        mlp[0:64, l * 128:l * 128 + 64] = wT
        mlp[64:128, l * 128 + 64:l * 128 + 128] = wT
    # final lhsT: diag(out_w^T, out_w^T) -> [128, 6]
    fin = np.zeros((128, 6), np.float32)
    fin[0:64, 0:3] = out_w.T
    fin[64:128, 3:6] = out_w.T
    eb = np.concatenate([be, be], axis=1).T.astype(np.float32)    # [128, 5]
    sb = np.concatenate([bs, bs], axis=1).T.astype(np.float32)    # [128, 5]
    mb = np.concatenate([lin_b, lin_b], axis=1).T.astype(np.float32)  # [128, 4]
    ob = np.concatenate([out_b, out_b]).reshape(1, 6).astype(np.float32)
    ones = np.ones((1, C), np.float32)
    return dict(gab=gab, mlp=mlp, fin=fin, eb=eb, sb=sb, mb=mb, ob=ob, ones=ones)


def _build_nc():
    import concourse.mybir as mybir
    import concourse.tile as tile
    from concourse import bacc

    f32 = mybir.dt.float32
    f32r = mybir.dt.float32r
    AF = mybir.ActivationFunctionType
    ALU = mybir.AluOpType

    nc = bacc.Bacc("TRN2", target_bir_lowering=False, debug=False,
                   enable_asserts=False, num_devices=NCORES)

    xs = nc.dram_tensor("xs", [2, NPIX], f32r, kind="ExternalInput").ap()
    gab_d = nc.dram_tensor("gab", [10, 10 * 128], f32r, kind="ExternalInput").ap()
    mlp_d = nc.dram_tensor("mlp", [128, 4 * 128], f32r, kind="ExternalInput").ap()
    fin_d = nc.dram_tensor("fin", [128, 6], f32r, kind="ExternalInput").ap()
    eb_d = nc.dram_tensor("eb", [128, 5], f32, kind="ExternalInput").ap()
    sb_d = nc.dram_tensor("sb", [128, 5], f32, kind="ExternalInput").ap()
    mb_d = nc.dram_tensor("mb", [128, 4], f32, kind="ExternalInput").ap()
    ob_d = nc.dram_tensor("ob", [1, 6], f32r, kind="ExternalInput").ap()
    ones_d = nc.dram_tensor("ones", [1, C], f32r, kind="ExternalInput").ap()
    out_d = nc.dram_tensor("out", [3, NPIX], f32, kind="ExternalOutput").ap()

    def mm_pair(psum_ap, lhsT_ap, rhs_ap):
        for h in range(C // MMCHUNK):
            sl = slice(h * MMCHUNK, (h + 1) * MMCHUNK)
            nc.tensor.matmul(out=psum_ap[:, sl],
                             lhsT=lhsT_ap, rhs=rhs_ap[:, sl],
                             start=True, stop=True)

    with tile.TileContext(nc) as tc:
        with (
            tc.tile_pool(name="consts", bufs=1) as cpool,
            tc.tile_pool(name="feat", bufs=2) as fpool,
            tc.tile_pool(name="pg", bufs=2, space="PSUM") as pg,
            tc.tile_pool(name="pm", bufs=2, space="PSUM") as pm,
            tc.tile_pool(name="epool", bufs=5 * ST + 2) as epool,
            tc.tile_pool(name="spool", bufs=4) as spool,
            tc.tile_pool(name="gpool", bufs=6) as gpool,
            tc.tile_pool(name="opool", bufs=4) as opool,
            tc.tile_pool(name="obuf", bufs=3) as obpool,
        ):
            gab = cpool.tile([10, 10 * 128], f32r)
            nc.sync.dma_start(out=gab, in_=gab_d)
            mlp = cpool.tile([128, 4 * 128], f32r)
            nc.sync.dma_start(out=mlp, in_=mlp_d)
            fin = cpool.tile([128, 6], f32r)
            nc.sync.dma_start(out=fin, in_=fin_d)
            eb = cpool.tile([128, 5], f32)
            nc.sync.dma_start(out=eb, in_=eb_d)
            sb = cpool.tile([128, 5], f32)
            nc.sync.dma_start(out=sb, in_=sb_d)
            mb = cpool.tile([128, 4], f32)
            nc.sync.dma_start(out=mb, in_=mb_d)
            ob = cpool.tile([1, 6], f32r)
            nc.sync.dma_start(out=ob, in_=ob_d)
            ones = cpool.tile([1, C], f32r)
            nc.sync.dma_start(out=ones, in_=ones_d)

            pending = []

            def flush_finals():
                for t, cur in pending:
                    pf = pm.tile([128, C], f32, tag="lin")
                    for h in range(C // MMCHUNK):
                        sl = slice(h * MMCHUNK, (h + 1) * MMCHUNK)
                        nc.tensor.matmul(out=pf[0:6, sl], lhsT=ob,
                                         rhs=ones[:, sl], start=True,
                                         stop=False)
                        nc.tensor.matmul(out=pf[0:6, sl], lhsT=fin,
                                         rhs=cur[:, sl], start=False,
                                         stop=True)
                    osb = obpool.tile([6, C], f32, tag="osb")
                    # Identity lives in the exp table set; flushed during the
                    # Exp phase so no extra ACT table switches occur.
                    nc.scalar.activation(out=osb, in_=pf[0:6],
                                         func=AF.Identity, bias=0.0)
                    nc.sync.dma_start(out=out_d[:, t * T:t * T + C],
                                      in_=osb[0:3])
                    nc.sync.dma_start(out=out_d[:, t * T + C:(t + 1) * T],
                                      in_=osb[3:6])
                pending.clear()

            for st in range(NT // ST):
                # --- features for the whole super-tile ---------------------
                # Compute ops require all operands to start on the same
                # partition. Scratch tile: partitions = pixel group (A,B),
                # free dims = (feature j, col): j 0:x1 1:x2 2:x1^2 3:x2^2
                # 4:x1x2. Products are computed in-tile at partition 0, then
                # two contiguous SBUF->SBUF DMAs build the [10, C] K-block
                # per tile (rows 0-4 = A feats, 5-9 = B feats).
                feat = fpool.tile([10, ST, C], f32r)
                scr = []
                for tt in range(ST):
                    t = st * ST + tt
                    s2 = fpool.tile([2, 5, C], f32r, tag="scr")
                    xtv = xs[:, t * T:(t + 1) * T]
                    xtv = xtv.rearrange("c (g p) -> g c p", p=C)
                    nc.sync.dma_start(out=s2[:, 0:2], in_=xtv)
                    nc.vector.tensor_mul(out=s2[:, 2:4], in0=s2[:, 0:2],
                                         in1=s2[:, 0:2])
                    nc.vector.tensor_mul(out=s2[:, 4], in0=s2[:, 0],
                                         in1=s2[:, 1])
                    nc.sync.dma_start(out=feat[0:5, tt],
                                      in_=s2[0:1].rearrange("p f c -> p (f c)"))
                    nc.sync.dma_start(out=feat[5:10, tt],
                                      in_=s2[1:2].rearrange("p f c -> p (f c)"))
                    scr.append(s2)

                # --- all Exp activations (one ACT table set) ---------------
                es = {}
                for tt in range(ST):
                    rhs = feat[0:10, tt]
                    for l in range(5):
                        ps = pg.tile([128, C], f32, tag="parg")
                        mm_pair(ps, gab[:, l * 128:(l + 1) * 128], rhs)
                        e = epool.tile([128, C], f32, tag="e")
                        nc.scalar.activation(out=e, in_=ps, func=AF.Exp,
                                             bias=eb[:, l:l + 1])
                        es[(tt, l)] = e

                flush_finals()

                # --- Sin + gabor product + MLP chain per tile --------------
                for tt in range(ST):
                    t = st * ST + tt
                    rhs = feat[0:10, tt]
                    g_tiles = []
                    for l in range(5):
                        ps = pg.tile([128, C], f32, tag="parg")
                        mm_pair(ps, gab[:, (5 + l) * 128:(6 + l) * 128], rhs)
                        s = spool.tile([128, C], f32, tag="s")
                        nc.scalar.activation(out=s, in_=ps, func=AF.Sin,
                                             bias=sb[:, l:l + 1])
                        g = gpool.tile([128, C], f32r, tag="g")
                        nc.vector.tensor_mul(out=g, in0=es[(tt, l)], in1=s)
                        g_tiles.append(g)

                    cur = g_tiles[0]
                    for l in range(1, 5):
                        pl = pm.tile([128, C], f32, tag="lin")
                        mm_pair(pl, mlp[:, (l - 1) * 128:l * 128], cur)
                        nxt = opool.tile([128, C], f32r, tag="o")
                        nc.vector.scalar_tensor_tensor(
                            out=nxt, in0=pl, scalar=mb[:, l - 1:l],
                            in1=g_tiles[l], op0=ALU.add, op1=ALU.mult)
                        cur = nxt

                    pending.append((t, cur))

            flush_finals()
    nc.compile()
    return nc


def _get_nc():
    if "nc" not in _CACHE:
        _CACHE["nc"] = _build_nc()
    return _CACHE["nc"]


def _in_maps(x, consts):
    maps = []
    rows = H // (NCORES // B)  # 128 rows per core
    for k in range(NCORES):
        b, r = k // (NCORES // B), (k % (NCORES // B)) * rows
        m = {"xs": np.ascontiguousarray(
            x[b, :, r:r + rows, :].reshape(2, NPIX), np.float32)}
        m.update(consts)
        maps.append(m)
    return maps


def _assemble(results):
    rows = H // (NCORES // B)
    out = np.empty((B, OUT, H, W), np.float32)
    for k in range(NCORES):
        b, r = k // (NCORES // B), (k % (NCORES // B)) * rows
        out[b, :, r:r + rows, :] = results[k]["out"].reshape(OUT, rows, W)
    return out


def run(x, filt_w, filt_b, mu, gamma, theta, lin_w, lin_b, out_w, out_b,
        trace=False):
    from concourse.bass_utils import run_bass_kernel_spmd
    nc = _get_nc()
    consts = _build_consts(np.asarray(filt_w), np.asarray(filt_b),
                           np.asarray(mu), np.asarray(gamma),
                           np.asarray(theta), np.asarray(lin_w),
                           np.asarray(lin_b), np.asarray(out_w),
                           np.asarray(out_b))
    maps = _in_maps(np.asarray(x), consts)
    res = run_bass_kernel_spmd(nc, maps, core_ids=list(range(NCORES)),
                               trace=trace)
    return _assemble(res.results), res


def kernel(**inputs):
    out, _ = run(**inputs)
    return out
